# revision 18
# baseline (speedup 1.0000x reference)
"""Trainium2 Bass kernel for the additive-attention glimpse module.

Math (per batch b):
    qp  = query @ Wq.T + bq                       # [E]
    cp  = context @ Wc.T + bc                     # [N, E]
    comb = tanh(qp + cp)                          # [N, E]
    attn = comb @ Wo.T (+ bo, softmax-invariant)  # [N, G]
    w    = softmax(attn, axis=N)                  # [N, G]
    out  = (w.T @ context).reshape(G*Cd)          # [G*Cd]

Shapes: B=256, N=196, Cd=2048, Qd=E=1024, G=8.

Strategy: data-parallel over B across 8 cores (32 batches each). The
dominant matmul (context @ Wc.T, ~26 GFLOP/core) runs feature-on-partition
(cp.T[e, r] = WcT.T @ ctx.T) with a mixed-precision contraction: channels
0:1024 in bf16 (1 row/cycle) and channels 1024:2048 in fp8-e4m3 with
perf_mode=DoubleRow (2 channels/cycle), keeping the end-to-end rel err
~1.6e-2 (< 2e-2 gate; pure fp8 measures 2.27e-2). ctx.T slabs are
pre-transposed on the host (the fp8 path cannot use the bf16-only DMA
xbar transpose). The softmax-weight transpose runs on the DMA xbar
(SBUF->SBUF), and the glimpse matmul for slab s-1 is emitted after the
cp/attn matmuls of slab s so the PE never waits on the softmax chain.
"""

import numpy as np
import ml_dtypes

BF16 = ml_dtypes.bfloat16
F8E4 = ml_dtypes.float8_e4m3  # TRN FP8_EXP4 grid (max +-240)

B_FULL = 256
N_CTX = 196
CD = 2048
QD = 1024
E = 1024
G = 8
N_CORES = 8
B_LOC = B_FULL // N_CORES  # 32

CB = 768                  # contraction channels done in bf16
CF = CD - CB              # contraction channels done in fp8 DoubleRow
NB = CB // 128            # bf16 k-tiles
NFP = CF // 256           # fp8 DoubleRow k-tile pairs

SLAB_B = 4                 # batches per slab
CHUNK_B = 2                # batches per compute chunk
CHUNK_R = CHUNK_B * N_CTX  # 392 (<=512 psum bank)


def build_nc(b_loc=B_LOC, reps=1, rep_scales=None, probe=None, **_legacy):
    """Build the single-core Bass/Tile graph (SPMD: same graph on all cores).

    reps>1 repeats the whole computation (same inputs -> same outputs)
    inside one NEFF; used only for wall-clock HW timing, since per-execute
    RPC overhead in this container is ~100ms.
    """
    import concourse.mybir as mybir
    import concourse.tile as tile
    from concourse import bacc
    from concourse.masks import make_identity

    f32 = mybir.dt.float32
    bf16 = mybir.dt.bfloat16
    fp8 = mybir.dt.float8e4
    Act = mybir.ActivationFunctionType
    Alu = mybir.AluOpType
    DR = mybir.MatmulPerfMode.DoubleRow

    assert b_loc % SLAB_B == 0
    n_slab = b_loc // SLAB_B
    R = b_loc * N_CTX

    nc = bacc.Bacc("TRN2", target_bir_lowering=False, debug=False,
                   num_devices=N_CORES)

    ctx = nc.dram_tensor("ctx", [R, CD], bf16, kind="ExternalInput").ap()
    xtb = nc.dram_tensor("xtb", [CB, R], bf16, kind="ExternalInput").ap()
    xt8 = nc.dram_tensor("xt8", [CF, R], fp8, kind="ExternalInput").ap()
    qT = nc.dram_tensor("qT", [QD, b_loc], bf16, kind="ExternalInput").ap()
    WqT = nc.dram_tensor("WqT", [QD, E], bf16, kind="ExternalInput").ap()
    WcTb = nc.dram_tensor("WcTb", [CB, E], bf16, kind="ExternalInput").ap()
    Wc8 = nc.dram_tensor("Wc8", [CF, E], fp8, kind="ExternalInput").ap()
    WoT = nc.dram_tensor("WoT", [E, G], bf16, kind="ExternalInput").ap()
    bqc = nc.dram_tensor("bqc", [128, E // 128], f32, kind="ExternalInput").ap()
    out = nc.dram_tensor("out", [b_loc, G * CD], f32, kind="ExternalOutput").ap()

    NE = E // 128    # 8 e-tiles
    NQ = QD // 128   # 8 q-tiles

    with tile.TileContext(nc) as tc:
        with (
            tc.tile_pool(name="const", bufs=1) as const_pool,
            tc.tile_pool(name="xtb", bufs=2) as xtb_pool,
            tc.tile_pool(name="xt8", bufs=2) as xt8_pool,
            tc.tile_pool(name="nat", bufs=2) as nat_pool,
            tc.tile_pool(name="comb", bufs=3) as comb_pool,
            tc.tile_pool(name="wex", bufs=8) as wex_pool,
            tc.tile_pool(name="sm", bufs=16) as sm_pool,
            tc.tile_pool(name="wl", bufs=16) as wl_pool,
            tc.tile_pool(name="outb", bufs=2) as outb_pool,
            tc.tile_pool(name="pcp",
                         bufs=(8 if probe in ("cpseq8", "nodma") else 4),
                         space="PSUM") as pc_pool,
            tc.tile_pool(name="pat", bufs=2, space="PSUM") as pa_pool,
            tc.tile_pool(name="pgl", bufs=2, space="PSUM") as pg_pool,
        ):
            # ---- persistent constants ----
            wcb_sb = const_pool.tile([128, NB, E], bf16)
            nc.sync.dma_start(wcb_sb[:], WcTb.rearrange("(k p) e -> p k e", p=128))
            wc8_sb = const_pool.tile([128, 2 * NFP, E], fp8)
            nc.sync.dma_start(wc8_sb[:], Wc8.rearrange("(k p) e -> p k e", p=128))
            wo_sb = const_pool.tile([128, NE, G], bf16)
            nc.sync.dma_start(wo_sb[:], WoT.rearrange("(k p) g -> p k g", p=128))
            bqc_sb = const_pool.tile([128, NE], f32)
            nc.sync.dma_start(bqc_sb[:], bqc[:])

            qpb_sb = const_pool.tile([128, NE, b_loc], f32, tag="qpb")
            ident = const_pool.tile([b_loc, b_loc], bf16)
            make_identity(nc, ident[:])

            def one_pass(out_scale=1.0):
                # ---- qp = query @ Wq.T (+bq+bc) ----
                # computed as [b, e] (qt stationary, WqT moving, N=512) then
                # PE-transposed per e-tile into the [e, b] bias layout
                qt_sb = wq_pool.tile([128, NQ, b_loc], bf16, tag="qt")
                nc.sync.dma_start(qt_sb[:], qT.rearrange("(k p) b -> p k b", p=128))
                wqm_sb = wq_pool.tile([128, NQ, E], bf16, tag="wqm")
                nc.sync.dma_start(
                    wqm_sb[:], WqT.rearrange("(k p) e -> p k e", p=128))
                qsb = wq_pool.tile([b_loc, E], bf16, tag="qsb")
                for half in range(2):
                    pqt = pc_pool.tile([b_loc, 512], f32, tag="pcp")
                    for k in range(NQ):
                        nc.tensor.matmul(
                            pqt[:], qt_sb[:, k, :],
                            wqm_sb[:, k, half * 512:(half + 1) * 512],
                            start=(k == 0), stop=(k == NQ - 1),
                        )
                    nc.scalar.activation(
                        qsb[:, half * 512:(half + 1) * 512], pqt[:], Act.Copy)
                for e in range(NE):
                    pt = pa_pool.tile([128, b_loc], bf16, tag="pat")
                    nc.tensor.transpose(
                        pt[:], qsb[:, e * 128:(e + 1) * 128], ident[:])
                    nc.vector.tensor_scalar_add(
                        qpb_sb[:, e, :], pt[:], bqc_sb[:, e:e + 1])

                prev = None  # deferred glimpse state of slab s-1

                def emit_glimpse(st):
                    # glimpse for 4 batches concurrently via PE column tiling:
                    # batch j occupies column group j (out parts 32j..32j+7)
                    wls, rss, nat_a, nat_b, s = st
                    outb = outb_pool.tile([128, CD], f32, tag="outb")
                    for cc in range(CD // 512):
                        pg = pg_pool.tile([128, 512], f32, tag="pgl")
                        for j in range(SLAB_B):
                            co = wls[j][2]
                            nc.tensor.matmul(
                                pg[32 * j:32 * j + G, :],
                                wls[j][0][:, co:co + G],
                                nat_a[:, j, cc * 512:(cc + 1) * 512],
                                start=True, stop=False,
                                tile_position=(0, 32 * j),
                                skip_group_check=True)
                        for j in range(SLAB_B):
                            co = wls[j][2]
                            nc.tensor.matmul(
                                pg[32 * j:32 * j + G, :],
                                wls[j][1][0:N_CTX - 128, co:co + G],
                                nat_b[:, j, cc * 512:(cc + 1) * 512],
                                start=False, stop=True,
                                tile_position=(0, 32 * j),
                                skip_group_check=True)
                        for j in range(SLAB_B):
                            dst = outb[32 * j:32 * j + G,
                                       cc * 512:(cc + 1) * 512]
                            if j % 2 == 0:
                                nc.vector.tensor_scalar_mul(
                                    dst, pg[32 * j:32 * j + G, :], rss[j][:])
                            else:
                                nc.scalar.activation(
                                    dst, pg[32 * j:32 * j + G, :],
                                    Act.Identity, bias=0.0, scale=rss[j][:])
                    for j in range(SLAB_B):
                        nc.gpsimd.dma_start(
                            out[s * SLAB_B + j, :].rearrange(
                                "(g c) -> g c", g=G),
                            outb[32 * j:32 * j + G, :])

                # ---- main loop over 4-batch slabs ----
                for s in range(n_slab):
                    r0 = s * SLAB_B * N_CTX
                    SR = SLAB_B * N_CTX
                    if probe == "nodma" and s > 0:
                        pass  # reuse slab-0 tiles
                    else:
                        xb = xtb_pool.tile([128, NB, SR], bf16, tag="xtb")
                        nc.sync.dma_start(
                            xb[:], xtb[:, r0:r0 + SR].rearrange(
                                "(k p) r -> p k r", p=128))
                        x8 = xt8_pool.tile([128, 2 * NFP, SR], fp8, tag="xt8")
                        nc.sync.dma_start(
                            x8[:], xt8[:, r0:r0 + SR].rearrange(
                                "(k p) r -> p k r", p=128))

                    if probe is None or probe == "noglim":
                        nat_a = nat_pool.tile([128, SLAB_B, CD], bf16,
                                              tag="nat_a")
                        nat_b = nat_pool.tile([N_CTX - 128, SLAB_B, CD], bf16,
                                              tag="nat_b")
                        slab_rows = ctx[r0:r0 + SR].rearrange(
                            "(j n) c -> n j c", j=SLAB_B)
                        nc.sync.dma_start(nat_a[:], slab_rows[0:128])
                        nc.sync.dma_start(nat_b[:], slab_rows[128:N_CTX])

                    # cp.T for both chunks, mixed bf16 + fp8-DoubleRow k-loop,
                    # weights paired across the two chunks
                    combs = [comb_pool.tile([128, NE, CHUNK_R], bf16,
                                            tag="comb", name=f"comb{h}")
                             for h in range(2)]
                    for e in range(NE):
                        es = slice(e * 128, (e + 1) * 128)
                        if probe == "ldwfix":
                            es = slice(0, 128)
                        pcs = [pc_pool.tile([128, CHUNK_R], f32,
                                            tag="pcp", name=f"pc{h}")
                               for h in range(2)]
                        if probe == "cpbf":
                            # pure bf16 contraction (16 k-tiles, sequential
                            # per chunk)
                            for h in range(2):
                                for c in range(16):
                                    nc.tensor.matmul(
                                        pcs[h][:], wcb_sb[:, c % NB, es],
                                        xb[:, c % NB,
                                           h * CHUNK_R:(h + 1) * CHUNK_R],
                                        start=(c == 0), stop=(c == 15),
                                    )
                        elif probe == "cpf8":
                            # pure fp8 DoubleRow contraction (8 pairs)
                            for h in range(2):
                                for kk in range(8):
                                    nc.tensor.matmul(
                                        pcs[h][:],
                                        wc8_sb[:, 2 * (kk % NFP):
                                               2 * (kk % NFP) + 2, es],
                                        x8[:, 2 * (kk % NFP):
                                           2 * (kk % NFP) + 2,
                                           h * CHUNK_R:(h + 1) * CHUNK_R],
                                        start=(kk == 0), stop=(kk == 7),
                                        perf_mode=DR,
                                    )
                        elif probe in ("cpseq", "cpseq8", "nodma"):
                            # mixed dtypes but no h-interleave
                            for h in range(2):
                                for c in range(NB):
                                    nc.tensor.matmul(
                                        pcs[h][:], wcb_sb[:, c, es],
                                        xb[:, c,
                                           h * CHUNK_R:(h + 1) * CHUNK_R],
                                        start=(c == 0), stop=False,
                                    )
                                for kk in range(NFP):
                                    nc.tensor.matmul(
                                        pcs[h][:],
                                        wc8_sb[:, 2 * kk:2 * kk + 2, es],
                                        x8[:, 2 * kk:2 * kk + 2,
                                           h * CHUNK_R:(h + 1) * CHUNK_R],
                                        start=False, stop=(kk == NFP - 1),
                                        perf_mode=DR,
                                    )
                        else:
                            for h in range(2):
                                for c in range(NB):
                                    cw = 0 if probe == "ldwfix" else c
                                    nc.tensor.matmul(
                                        pcs[h][:], wcb_sb[:, cw, es],
                                        xb[:, c,
                                           h * CHUNK_R:(h + 1) * CHUNK_R],
                                        start=(c == 0), stop=False,
                                    )
                                for kk in range(NFP):
                                    kw = 0 if probe == "ldwfix" else kk
                                    nc.tensor.matmul(
                                        pcs[h][:],
                                        wc8_sb[:, 2 * kw:2 * kw + 2, es],
                                        x8[:, 2 * kk:2 * kk + 2,
                                           h * CHUNK_R:(h + 1) * CHUNK_R],
                                        start=False, stop=(kk == NFP - 1),
                                        perf_mode=DR,
                                    )
                        if probe in ("cp", "ldwfix", "nodma", "cpbf", "cpf8",
                                     "cpseq", "cpseq8"):
                            continue
                        for h in range(2):
                            b0 = s * SLAB_B + h * CHUNK_B
                            for j in range(CHUNK_B):
                                nc.scalar.activation(
                                    combs[h][:, e, j * N_CTX:(j + 1) * N_CTX],
                                    pcs[h][:, j * N_CTX:(j + 1) * N_CTX],
                                    Act.Tanh,
                                    bias=qpb_sb[:, e, b0 + j:b0 + j + 1],
                                )

                    if probe in ("cp", "ldwfix", "nodma", "cpbf", "cpf8",
                                 "cpseq", "cpseq8"):
                        ob = outb_pool.tile([128, CHUNK_R], f32, tag="outb")
                        nc.scalar.activation(ob[:], pcs[0][:], Act.Copy)
                        nc.gpsimd.dma_start(
                            out[s * SLAB_B, 0:CHUNK_R].rearrange(
                                "(g c) -> g c", g=1),
                            ob[0:1, :])
                        continue

                    # attn.T = WoT.T @ comb.T; both batches of a chunk in
                    # one mm (out [G, 392])
                    pa_tiles = []
                    for h in range(2):
                        pa = pa_pool.tile([G, CHUNK_R], f32, tag="pat")
                        for e in range(NE):
                            nc.tensor.matmul(
                                pa[:], wo_sb[:, e, :], combs[h][:, e, :],
                                start=(e == 0), stop=(e == NE - 1),
                            )
                        pa_tiles.append(pa)

                    # softmax (no max-subtraction: logits are O(+-5)) and
                    # weight transpose via the DMA xbar; the 2 batches of a
                    # chunk share one [32, 256] exp tile and one transpose
                    wls, rss = [], []
                    for h in range(2):
                        pa = pa_tiles[h]
                        wex = wex_pool.tile([64, 256], bf16, tag="wex")
                        nc.vector.memset(wex[:], 0.0)
                        for jj in range(CHUNK_B):
                            seg = pa[:, jj * N_CTX:(jj + 1) * N_CTX]
                            ssum = sm_pool.tile([G, 1], f32, tag="ssum")
                            nc.scalar.activation(
                                wex[32 * jj:32 * jj + G, 0:N_CTX], seg,
                                Act.Exp, accum_out=ssum[:])
                            rs = sm_pool.tile([G, 1], f32, tag="rs")
                            nc.vector.reciprocal(rs[:], ssum[:])
                            if out_scale != 1.0:
                                nc.vector.tensor_scalar_mul(
                                    rs[:], rs[:], float(out_scale))
                            rss.append(rs)
                        wla = wl_pool.tile([128, 64], bf16, tag="wla")
                        wlb = wl_pool.tile([128, 64], bf16, tag="wlb")
                        nc.sync.dma_start_transpose(wla[:], wex[:, 0:128])
                        nc.sync.dma_start_transpose(wlb[:], wex[:, 128:256])
                        for jj in range(CHUNK_B):
                            wls.append((wla, wlb, 32 * jj))

                    if probe == "noglim":
                        ob = outb_pool.tile([128, N_CTX], f32, tag="outb")
                        nc.scalar.activation(ob[:], pa_tiles[0][:], Act.Copy)
                        nc.gpsimd.dma_start(
                            out[s * SLAB_B, 0:N_CTX].rearrange(
                                "(g c) -> g c", g=1),
                            ob[0:1, :])
                        continue

                    if prev is not None:
                        emit_glimpse(prev)
                    prev = (wls, rss, nat_a, nat_b, s)

                if probe is None:
                    emit_glimpse(prev)

            with tc.tile_pool(name="wq", bufs=1) as wq_pool:
                for _rep in range(reps):
                    one_pass(out_scale=rep_scales[_rep] if rep_scales else 1.0)

    nc.compile()
    return nc


_NC_CACHE = {}


def _get_nc(b_loc=B_LOC):
    if b_loc not in _NC_CACHE:
        _NC_CACHE[b_loc] = build_nc(b_loc)
    return _NC_CACHE[b_loc]


def make_in_maps(context, query, Wq, bq, Wc, bc, Wo, bo, b_loc=B_LOC,
                 n_cores=N_CORES):
    """Host-side prep: dtype conversion, weight/context transposes, sharding."""
    context = np.asarray(context)
    query = np.asarray(query)
    Wq, bq = np.asarray(Wq), np.asarray(bq)
    Wc, bc = np.asarray(Wc), np.asarray(bc)
    Wo = np.asarray(Wo)
    ctx_bf = np.ascontiguousarray(context).astype(BF16)
    ctx_f = context.reshape(B_FULL * N_CTX, CD)
    ctxT_b = np.ascontiguousarray(ctx_f.T[:CB]).astype(BF16)
    ctxT_8 = np.ascontiguousarray(ctx_f.T[CB:]).astype(F8E4)
    WqT = np.ascontiguousarray(Wq.T).astype(BF16)
    WcT = Wc.T
    WcTb = np.ascontiguousarray(WcT[:CB]).astype(BF16)
    Wc8 = np.ascontiguousarray(WcT[CB:]).astype(F8E4)
    WoT = np.ascontiguousarray(Wo.T).astype(BF16)
    bqc = np.ascontiguousarray(
        (bq + bc).astype(np.float32).reshape(E // 128, 128).T)
    in_maps = []
    for i in range(n_cores):
        b0 = i * b_loc
        r0 = b0 * N_CTX
        in_maps.append(dict(
            ctx=ctx_bf[b0:b0 + b_loc].reshape(b_loc * N_CTX, CD),
            xtb=np.ascontiguousarray(ctxT_b[:, r0:r0 + b_loc * N_CTX]),
            xt8=np.ascontiguousarray(ctxT_8[:, r0:r0 + b_loc * N_CTX]),
            qT=np.ascontiguousarray(query[b0:b0 + b_loc].T.astype(BF16)),
            WqT=WqT, WcTb=WcTb, Wc8=Wc8, WoT=WoT, bqc=bqc,
        ))
    return in_maps


def kernel(context, query, Wq, bq, Wc, bc, Wo, bo):
    from concourse.bass_utils import run_bass_kernel_spmd

    assert context.shape == (B_FULL, N_CTX, CD)
    nc = _get_nc()
    in_maps = make_in_maps(context, query, Wq, bq, Wc, bc, Wo, bo)
    res = run_bass_kernel_spmd(nc, in_maps, core_ids=list(range(N_CORES)))
    return np.concatenate([res.results[i]["out"] for i in range(N_CORES)],
                          axis=0)


# revision 21
# speedup vs baseline: 1.0549x; 1.0549x over previous
"""Trainium2 Bass kernel for the additive-attention glimpse module.

Math (per batch b):
    qp  = query @ Wq.T + bq                       # [E]
    cp  = context @ Wc.T + bc                     # [N, E]
    comb = tanh(qp + cp)                          # [N, E]
    attn = comb @ Wo.T (+ bo, softmax-invariant)  # [N, G]
    w    = softmax(attn, axis=N)                  # [N, G]
    out  = (w.T @ context).reshape(G*Cd)          # [G*Cd]

Shapes: B=256, N=196, Cd=2048, Qd=E=1024, G=8.

Strategy: data-parallel over B across 8 cores (32 batches each). The
dominant matmul (context @ Wc.T) runs feature-on-partition with a
mixed-precision contraction: channels 0:768 in bf16, channels 768:2048 in
fp8-e4m3 with perf_mode=DoubleRow (K=256/instruction), end-to-end rel err
~1.79e-2 (< 2e-2 gate). The kernel is per-PE-instruction-bound (~200ns/mm
floor), so cp+attn are gridded in 512-row chunks that span batch/slab
boundaries (13 chunks instead of 16 per e-tile); attn logits are copied
into a per-slab SBUF buffer so softmax stays batch-contiguous. The
softmax-weight transpose runs on the DMA xbar (SBUF->SBUF), and the
glimpse matmul for slab s-1 is emitted after slab s completes so the PE
never waits on the softmax chain.
"""

import numpy as np
import ml_dtypes

BF16 = ml_dtypes.bfloat16
F8E4 = ml_dtypes.float8_e4m3  # TRN FP8_EXP4 grid (max +-240)

B_FULL = 256
N_CTX = 196
CD = 2048
QD = 1024
E = 1024
G = 8
N_CORES = 8
B_LOC = B_FULL // N_CORES  # 32

CB = 768                  # contraction channels done in bf16
CF = CD - CB              # contraction channels done in fp8 DoubleRow
NB = CB // 128            # bf16 k-tiles
NFP = CF // 256           # fp8 DoubleRow k-tile pairs

SLAB_B = 4                 # batches per glimpse slab
SR = SLAB_B * N_CTX        # 784 rows per slab
CK = 512                   # cp/attn chunk rows (= one psum bank of f32)


def build_nc(b_loc=B_LOC, reps=1, rep_scales=None, probe=None, **_legacy):
    """Build the single-core Bass/Tile graph (SPMD: same graph on all cores).

    reps>1 repeats the whole computation (same inputs -> same outputs)
    inside one NEFF; used only for wall-clock HW timing, since per-execute
    RPC overhead in this container is ~100ms.
    """
    import concourse.mybir as mybir
    import concourse.tile as tile
    from concourse import bacc
    from concourse.masks import make_identity

    f32 = mybir.dt.float32
    bf16 = mybir.dt.bfloat16
    fp8 = mybir.dt.float8e4
    Act = mybir.ActivationFunctionType
    DR = mybir.MatmulPerfMode.DoubleRow

    assert b_loc % SLAB_B == 0
    n_slab = b_loc // SLAB_B
    R = b_loc * N_CTX
    n_chunk = (R + CK - 1) // CK  # 13 for b_loc=32

    nc = bacc.Bacc("TRN2", target_bir_lowering=False, debug=False,
                   num_devices=N_CORES)

    ctx = nc.dram_tensor("ctx", [R, CD], bf16, kind="ExternalInput").ap()
    xtb = nc.dram_tensor("xtb", [CB, R], bf16, kind="ExternalInput").ap()
    xt8 = nc.dram_tensor("xt8", [CF, R], fp8, kind="ExternalInput").ap()
    qT = nc.dram_tensor("qT", [QD, b_loc], bf16, kind="ExternalInput").ap()
    WqT = nc.dram_tensor("WqT", [QD, E], bf16, kind="ExternalInput").ap()
    WcTb = nc.dram_tensor("WcTb", [CB, E], bf16, kind="ExternalInput").ap()
    Wc8 = nc.dram_tensor("Wc8", [CF, E], fp8, kind="ExternalInput").ap()
    WoT = nc.dram_tensor("WoT", [E, G], bf16, kind="ExternalInput").ap()
    bqc = nc.dram_tensor("bqc", [128, E // 128], f32, kind="ExternalInput").ap()
    out = nc.dram_tensor("out", [b_loc, G * CD], f32, kind="ExternalOutput").ap()

    NE = E // 128    # 8 e-tiles
    NQ = QD // 128   # 8 q-tiles

    with tile.TileContext(nc) as tc:
        with (
            tc.tile_pool(name="const", bufs=1) as const_pool,
            tc.tile_pool(name="xtb", bufs=2) as xtb_pool,
            tc.tile_pool(name="xt8", bufs=2) as xt8_pool,
            tc.tile_pool(name="nat", bufs=2) as nat_pool,
            tc.tile_pool(name="comb", bufs=2) as comb_pool,
            tc.tile_pool(name="att", bufs=2) as att_pool,
            tc.tile_pool(name="wex", bufs=8) as wex_pool,
            tc.tile_pool(name="sm", bufs=16) as sm_pool,
            tc.tile_pool(name="wl", bufs=16) as wl_pool,
            tc.tile_pool(name="outb", bufs=2) as outb_pool,
            tc.tile_pool(name="pcp", bufs=4, space="PSUM") as pc_pool,
            tc.tile_pool(name="pat", bufs=2, space="PSUM") as pa_pool,
            tc.tile_pool(name="pgl", bufs=2, space="PSUM") as pg_pool,
        ):
            # ---- persistent constants ----
            wcb_sb = const_pool.tile([128, NB, E], bf16)
            nc.sync.dma_start(wcb_sb[:], WcTb.rearrange("(k p) e -> p k e", p=128))
            wc8_sb = const_pool.tile([128, 2 * NFP, E], fp8)
            nc.sync.dma_start(wc8_sb[:], Wc8.rearrange("(k p) e -> p k e", p=128))
            wo_sb = const_pool.tile([128, NE, G], bf16)
            nc.sync.dma_start(wo_sb[:], WoT.rearrange("(k p) g -> p k g", p=128))
            bqc_sb = const_pool.tile([128, NE], f32)
            nc.sync.dma_start(bqc_sb[:], bqc[:])

            qpb_sb = const_pool.tile([128, NE, b_loc], f32, tag="qpb")
            ident = const_pool.tile([b_loc, b_loc], bf16)
            make_identity(nc, ident[:])

            def one_pass(out_scale=1.0):
                # ---- qp = query @ Wq.T (+bq+bc) ----
                # computed as [b, e] (qt stationary, WqT moving, N=512) then
                # PE-transposed per e-tile into the [e, b] bias layout
                qt_sb = wq_pool.tile([128, NQ, b_loc], bf16, tag="qt")
                nc.sync.dma_start(qt_sb[:], qT.rearrange("(k p) b -> p k b", p=128))
                wqm_sb = wq_pool.tile([128, NQ, E], bf16, tag="wqm")
                nc.sync.dma_start(
                    wqm_sb[:], WqT.rearrange("(k p) e -> p k e", p=128))
                qsb = wq_pool.tile([b_loc, E], bf16, tag="qsb")
                for half in range(2):
                    pqt = pc_pool.tile([b_loc, 512], f32, tag="pcp")
                    for k in range(NQ):
                        nc.tensor.matmul(
                            pqt[:], qt_sb[:, k, :],
                            wqm_sb[:, k, half * 512:(half + 1) * 512],
                            start=(k == 0), stop=(k == NQ - 1),
                        )
                    nc.scalar.activation(
                        qsb[:, half * 512:(half + 1) * 512], pqt[:], Act.Copy)
                for e in range(NE):
                    pt = pa_pool.tile([128, b_loc], bf16, tag="pat")
                    nc.tensor.transpose(
                        pt[:], qsb[:, e * 128:(e + 1) * 128], ident[:])
                    nc.vector.tensor_scalar_add(
                        qpb_sb[:, e, :], pt[:], bqc_sb[:, e:e + 1])

                prev = None       # deferred glimpse state of previous slab
                slab_state = {}   # s -> [None, None, att]
                nat_tiles = {}    # s -> (nat_a, nat_b)

                def emit_glimpse(st):
                    # glimpse for 4 batches concurrently via PE column tiling
                    wls, rss, nat_a, nat_b, s = st
                    outb = outb_pool.tile([128, CD], f32, tag="outb")
                    for cc in range(CD // 512):
                        pg = pg_pool.tile([128, 512], f32, tag="pgl")
                        for j in range(SLAB_B):
                            co = wls[j][2]
                            nc.tensor.matmul(
                                pg[32 * j:32 * j + G, :],
                                wls[j][0][:, co:co + G],
                                nat_a[:, j, cc * 512:(cc + 1) * 512],
                                start=True, stop=False,
                                tile_position=(0, 32 * j),
                                skip_group_check=True)
                        for j in range(SLAB_B):
                            co = wls[j][2]
                            nc.tensor.matmul(
                                pg[32 * j:32 * j + G, :],
                                wls[j][1][0:N_CTX - 128, co:co + G],
                                nat_b[:, j, cc * 512:(cc + 1) * 512],
                                start=False, stop=True,
                                tile_position=(0, 32 * j),
                                skip_group_check=True)
                        for j in range(SLAB_B):
                            dst = outb[32 * j:32 * j + G,
                                       cc * 512:(cc + 1) * 512]
                            if j % 2 == 0:
                                nc.vector.tensor_scalar_mul(
                                    dst, pg[32 * j:32 * j + G, :], rss[j][:])
                            else:
                                nc.scalar.activation(
                                    dst, pg[32 * j:32 * j + G, :],
                                    Act.Identity, bias=0.0, scale=rss[j][:])
                    for j in range(SLAB_B):
                        nc.gpsimd.dma_start(
                            out[s * SLAB_B + j, :].rearrange(
                                "(g c) -> g c", g=G),
                            outb[32 * j:32 * j + G, :])

                # ---- main loop over 512-row chunks (cross-slab) ----
                for t in range(n_chunk):
                    c0 = t * CK
                    cw = min(CK, R - c0)

                    # start-of-slab: allocate the logit buffer (nat DMA is
                    # emitted after s_done below, so the recycled nat buffer's
                    # previous readers are already in program order)
                    s_new = [s for s in range(n_slab)
                             if (s * SR) // CK == t]
                    for s in s_new:
                        att = att_pool.tile([G, SR], f32, tag="att")
                        slab_state[s] = [None, None, att]

                    xb = xtb_pool.tile([128, NB, CK], bf16, tag="xtb")
                    nc.sync.dma_start(
                        xb[:, :, 0:cw], xtb[:, c0:c0 + cw].rearrange(
                            "(k p) r -> p k r", p=128))
                    x8 = xt8_pool.tile([128, 2 * NFP, CK], fp8, tag="xt8")
                    nc.sync.dma_start(
                        x8[:, :, 0:cw], xt8[:, c0:c0 + cw].rearrange(
                            "(k p) r -> p k r", p=128))

                    # batch segments of this chunk (each contiguous)
                    b_lo = c0 // N_CTX
                    b_hi = (c0 + cw - 1) // N_CTX
                    segs = []
                    for b in range(b_lo, b_hi + 1):
                        g0 = max(c0, b * N_CTX)
                        g1 = min(c0 + cw, (b + 1) * N_CTX)
                        segs.append((b, g0 - c0, g1 - c0))

                    # cp.T: 6 bf16 + 5 fp8-DoubleRow k-instrs per e-tile
                    comb = comb_pool.tile([128, NE, CK], bf16, tag="comb")
                    for e in range(NE):
                        es = slice(e * 128, (e + 1) * 128)
                        pc = pc_pool.tile([128, CK], f32, tag="pcp")
                        for c in range(NB):
                            nc.tensor.matmul(
                                pc[:, 0:cw], wcb_sb[:, c, es],
                                xb[:, c, 0:cw],
                                start=(c == 0), stop=False,
                            )
                        for kk in range(NFP):
                            nc.tensor.matmul(
                                pc[:, 0:cw],
                                wc8_sb[:, 2 * kk:2 * kk + 2, es],
                                x8[:, 2 * kk:2 * kk + 2, 0:cw],
                                start=False, stop=(kk == NFP - 1),
                                perf_mode=DR,
                            )
                        for b, l0, l1 in segs:
                            nc.scalar.activation(
                                comb[:, e, l0:l1], pc[:, l0:l1],
                                Act.Tanh,
                                bias=qpb_sb[:, e, b:b + 1],
                            )

                    # attn.T = WoT.T @ comb.T for the whole chunk
                    pa = pa_pool.tile([G, CK], f32, tag="pat")
                    for e in range(NE):
                        nc.tensor.matmul(
                            pa[:, 0:cw], wo_sb[:, e, :], comb[:, e, 0:cw],
                            start=(e == 0), stop=(e == NE - 1),
                        )

                    # scatter logits into per-slab att buffers
                    s_lo = c0 // SR
                    s_hi = (c0 + cw - 1) // SR
                    for s in range(s_lo, s_hi + 1):
                        o0 = max(c0, s * SR)
                        o1 = min(c0 + cw, (s + 1) * SR)
                        nc.scalar.activation(
                            slab_state[s][2][:, o0 - s * SR:o1 - s * SR],
                            pa[:, o0 - c0:o1 - c0], Act.Copy)

                    # slabs fully covered by now: softmax + wl transpose,
                    # then emit the previous slab's glimpse
                    s_done = [s for s in range(n_slab)
                              if ((s + 1) * SR + CK - 1) // CK - 1 == t]
                    for s in s_done:
                        _, _, att = slab_state.pop(s)
                        nat_a, nat_b = nat_tiles.pop(s)
                        wls, rss = [], []
                        for h in range(2):
                            wex = wex_pool.tile([64, 256], bf16, tag="wex")
                            nc.vector.memset(wex[:], 0.0)
                            for jj in range(2):
                                j = 2 * h + jj
                                seg = att[:, j * N_CTX:(j + 1) * N_CTX]
                                ssum = sm_pool.tile([G, 1], f32, tag="ssum")
                                nc.scalar.activation(
                                    wex[32 * jj:32 * jj + G, 0:N_CTX], seg,
                                    Act.Exp, accum_out=ssum[:])
                                rs = sm_pool.tile([G, 1], f32, tag="rs")
                                nc.vector.reciprocal(rs[:], ssum[:])
                                if out_scale != 1.0:
                                    nc.vector.tensor_scalar_mul(
                                        rs[:], rs[:], float(out_scale))
                                rss.append(rs)
                            wla = wl_pool.tile([128, 64], bf16, tag="wla")
                            wlb = wl_pool.tile([128, 64], bf16, tag="wlb")
                            nc.sync.dma_start_transpose(wla[:], wex[:, 0:128])
                            nc.sync.dma_start_transpose(wlb[:],
                                                        wex[:, 128:256])
                            for jj in range(2):
                                wls.append((wla, wlb, 32 * jj))
                        if prev is not None:
                            emit_glimpse(prev)
                        prev = (wls, rss, nat_a, nat_b, s)

                    for s in s_new:
                        r0 = s * SR
                        nat_a = nat_pool.tile([128, SLAB_B, CD], bf16,
                                              tag="nat_a")
                        nat_b = nat_pool.tile([N_CTX - 128, SLAB_B, CD],
                                              bf16, tag="nat_b")
                        slab_rows = ctx[r0:r0 + SR].rearrange(
                            "(j n) c -> n j c", j=SLAB_B)
                        nc.sync.dma_start(nat_a[:], slab_rows[0:128])
                        nc.sync.dma_start(nat_b[:], slab_rows[128:N_CTX])
                        nat_tiles[s] = (nat_a, nat_b)

                emit_glimpse(prev)

            with tc.tile_pool(name="wq", bufs=1) as wq_pool:
                for _rep in range(reps):
                    one_pass(out_scale=rep_scales[_rep] if rep_scales else 1.0)

    nc.compile()
    return nc


_NC_CACHE = {}


def _get_nc(b_loc=B_LOC):
    if b_loc not in _NC_CACHE:
        _NC_CACHE[b_loc] = build_nc(b_loc)
    return _NC_CACHE[b_loc]


def make_in_maps(context, query, Wq, bq, Wc, bc, Wo, bo, b_loc=B_LOC,
                 n_cores=N_CORES):
    """Host-side prep: dtype conversion, weight/context transposes, sharding."""
    context = np.asarray(context)
    query = np.asarray(query)
    Wq, bq = np.asarray(Wq), np.asarray(bq)
    Wc, bc = np.asarray(Wc), np.asarray(bc)
    Wo = np.asarray(Wo)
    ctx_bf = np.ascontiguousarray(context).astype(BF16)
    ctx_f = context.reshape(B_FULL * N_CTX, CD)
    ctxT_b = np.ascontiguousarray(ctx_f.T[:CB]).astype(BF16)
    ctxT_8 = np.ascontiguousarray(ctx_f.T[CB:]).astype(F8E4)
    WqT = np.ascontiguousarray(Wq.T).astype(BF16)
    WcT = Wc.T
    WcTb = np.ascontiguousarray(WcT[:CB]).astype(BF16)
    Wc8 = np.ascontiguousarray(WcT[CB:]).astype(F8E4)
    WoT = np.ascontiguousarray(Wo.T).astype(BF16)
    bqc = np.ascontiguousarray(
        (bq + bc).astype(np.float32).reshape(E // 128, 128).T)
    in_maps = []
    for i in range(n_cores):
        b0 = i * b_loc
        r0 = b0 * N_CTX
        in_maps.append(dict(
            ctx=ctx_bf[b0:b0 + b_loc].reshape(b_loc * N_CTX, CD),
            xtb=np.ascontiguousarray(ctxT_b[:, r0:r0 + b_loc * N_CTX]),
            xt8=np.ascontiguousarray(ctxT_8[:, r0:r0 + b_loc * N_CTX]),
            qT=np.ascontiguousarray(query[b0:b0 + b_loc].T.astype(BF16)),
            WqT=WqT, WcTb=WcTb, Wc8=Wc8, WoT=WoT, bqc=bqc,
        ))
    return in_maps


def kernel(context, query, Wq, bq, Wc, bc, Wo, bo):
    from concourse.bass_utils import run_bass_kernel_spmd

    assert context.shape == (B_FULL, N_CTX, CD)
    nc = _get_nc()
    in_maps = make_in_maps(context, query, Wq, bq, Wc, bc, Wo, bo)
    res = run_bass_kernel_spmd(nc, in_maps, core_ids=list(range(N_CORES)))
    return np.concatenate([res.results[i]["out"] for i in range(N_CORES)],
                          axis=0)
